# revision 6
# baseline (speedup 1.0000x reference)
"""FacenetLoss Trainium2 kernel.

Strategy (v2)
-------------
N=384, D=128.  The reference builds an [N,N,N] triplet tensor; we never
materialize it.  For anchor i, positive j, negative k:

    tl[i,j,k] = relu((d_ij + MARGIN) - d_ik)

with d the squared-euclidean pairwise distance.  Since
d_ij - d_ik = (sq_j - 2 G_ij) - (sq_k - 2 G_ik)  (sq_i cancels, G = E @ E.T),
we only need on device:

    U[i,k] = sq_k - 2*G[i,k]     bias(i,j) = U[i,j] + M;  row term = -U[i,k]

Only same-class (i,j) pairs ever contribute (rows with same_group == 0 reduce
to exactly 0 in the reference), so we enumerate the ~N^2/C valid pairs on host
(integer bookkeeping only; all math derived from embeddings stays on device),
pack them 128 per partition-tile, and per pair-slot:

  - gather U[i,:] rows with ONE dma_gather (int16 idx table, 1536B rows)
  - gather U[i*N+j] bias elements with ONE multi-offset indirect DMA
  - add host-built poison rows (BIG where classes[k]==classes[i]) on GpSimd
    so invalid negatives k saturate the relu to exactly 0
  - t   = relu(bias + M - U_row - poison_row)       [ACT, per tile]
  - pps = max_k( t * (t <= M) )                     [DVE]
  - sh  = min_k( t - RHO*(t > M) )                  [DVE + GpSimd add]
  - ppl = w * ( pps + (pps==0) * (sh+RHO) * (sh<0) )

Per-core scalar partials are summed on host; loss = num / max(den,1) with den
the host-side valid-pair count.  Poisoned k give t == 0 exactly, which drops
out of pps (max of nonnegatives) and leaves the (sh<0)-gated pph unchanged.
"""

import functools
import math

import numpy as np

N = 384
D = 128
NB = N // 128  # 3 row blocks
P = 128
NCORES = 8
MARGIN = 0.2
RHO = 10.0
BIG = 1.0e5


@functools.lru_cache(maxsize=4)
def _build(T: int):
    """Compile the SPMD kernel for T pair-tiles per core. Returns nc."""
    from contextlib import ExitStack

    import concourse.bacc as bacc
    import concourse.bass as bass
    import concourse.mybir as mybir
    import concourse.tile as tile
    from concourse.masks import make_identity

    f32 = mybir.dt.float32
    i32 = mybir.dt.int32
    i16 = mybir.dt.int16
    Alu = mybir.AluOpType
    Act = mybir.ActivationFunctionType
    Ax = mybir.AxisListType

    NI = T * P              # pair slots per core
    NIC = (NI + 15) // 16   # idx table columns for dma_gather

    nc = bacc.Bacc("TRN2", target_bir_lowering=False, debug=False,
                   num_devices=NCORES)

    emb = nc.dram_tensor("emb", [N, D], f32, kind="ExternalInput").ap()
    rowidx = nc.dram_tensor("rowidx", [P, T], i32, kind="ExternalInput").ap()
    boff = nc.dram_tensor("boff", [P, T], i32, kind="ExternalInput").ap()
    wvec = nc.dram_tensor("wvec", [P, T], f32, kind="ExternalInput").ap()
    poirows = nc.dram_tensor("poirows", [P, T * N], f32,
                             kind="ExternalInput").ap()
    out = nc.dram_tensor("partial", [1, 1], f32, kind="ExternalOutput").ap()

    with tile.TileContext(nc) as tc, ExitStack() as ctx:
        sb = ctx.enter_context(tc.tile_pool(name="sb", bufs=1))
        work = ctx.enter_context(tc.tile_pool(name="work", bufs=3))
        ps = ctx.enter_context(tc.tile_pool(name="ps", bufs=2, space="PSUM"))
        dram = ctx.enter_context(tc.tile_pool(name="dram", bufs=1,
                                              space="DRAM"))

        # ---- Phase A: U = sqrow - 2G, straight to DRAM -------------------
        esb = sb.tile([P, NB * D], f32)
        for b in range(NB):
            nc.sync.dma_start(out=esb[:, b * D:(b + 1) * D],
                              in_=emb[b * P:(b + 1) * P, :])

        idt = sb.tile([P, P], f32)
        make_identity(nc, idt[:])

        # host tables (gpsimd queue, after identity so PE isn't stalled)
        ridx = sb.tile([P, T], i32)
        nc.gpsimd.dma_start(out=ridx[:], in_=rowidx[:, :])
        bofs = sb.tile([P, T], i32)
        nc.gpsimd.dma_start(out=bofs[:], in_=boff[:, :])
        wv = sb.tile([P, T], f32)
        nc.gpsimd.dma_start(out=wv[:], in_=wvec[:, :])
        poi = sb.tile([P, T, N], f32)
        nc.gpsimd.dma_start(out=poi[:], in_=poirows[:, :])

        # ET[d, n] = emb[n, d]  via PE transpose per block
        et = sb.tile([P, N], f32)
        for b in range(NB):
            pst = ps.tile([P, P], f32)
            nc.tensor.transpose(out=pst[:], in_=esb[:, b * D:(b + 1) * D],
                                identity=idt[:])
            nc.scalar.copy(out=et[:, b * P:(b + 1) * P], in_=pst[:])

        # sqrow[0, n] = sum_d emb[n, d]^2
        etsq = sb.tile([P, N], f32)
        nc.vector.tensor_tensor(out=etsq[:], in0=et[:], in1=et[:],
                                op=Alu.mult)
        ones = sb.tile([P, 1], f32)
        nc.vector.memset(ones[:], 1.0)
        ps_sq = ps.tile([1, N], f32)
        nc.tensor.matmul(out=ps_sq[:], lhsT=ones[:, 0:1], rhs=etsq[:],
                         start=True, stop=True)
        sqrow = sb.tile([1, N], f32)
        nc.scalar.copy(out=sqrow[:], in_=ps_sq[:])
        ones_r = sb.tile([1, P], f32)
        nc.vector.memset(ones_r[:], 1.0)
        etm2 = sb.tile([P, N], f32)
        nc.vector.tensor_scalar(out=etm2[:], in0=et[:], scalar1=-2.0,
                                scalar2=None, op0=Alu.mult)

        udram = dram.tile([N * N, 1], f32)
        ud2 = udram[:, :].rearrange("(a b) o -> a (b o)", b=N)
        for b in range(NB):
            ps_u = ps.tile([P, N], f32)
            nc.tensor.matmul(out=ps_u[:], lhsT=etm2[:, b * P:(b + 1) * P],
                             rhs=et[:], start=True, stop=False)
            nc.tensor.matmul(out=ps_u[:], lhsT=ones_r[0:1, :],
                             rhs=sqrow[0:1, :], start=False, stop=True)
            ub = work.tile([P, N], f32)
            nc.scalar.copy(out=ub[:], in_=ps_u[:])
            nc.sync.dma_start(out=ud2[b * P:(b + 1) * P, :], in_=ub[:])

        # ---- Phase B: per-tile gathers + triplet reductions -------------
        bval = work.tile([P, T], f32)
        urows = work.tile([P, T, N], f32)
        upoi = work.tile([P, T, N], f32)
        for t in range(T):
            nc.gpsimd.indirect_dma_start(
                out=urows[:, t, :], out_offset=None, in_=ud2,
                in_offset=bass.IndirectOffsetOnAxis(ap=ridx[:, t:t + 1],
                                                    axis=0))
            nc.gpsimd.indirect_dma_start(
                out=bval[:, t:t + 1], out_offset=None, in_=udram[:, :],
                in_offset=bass.IndirectOffsetOnAxis(ap=bofs[:, t:t + 1],
                                                    axis=0))
            nc.gpsimd.tensor_tensor(out=upoi[:, t, :], in0=urows[:, t, :],
                                    in1=poi[:, t, :], op=Alu.add)
        bm = work.tile([P, T], f32)
        nc.vector.tensor_scalar(out=bm[:], in0=bval[:],
                                scalar1=float(MARGIN), scalar2=None,
                                op0=Alu.add)

        pps = sb.tile([P, T], f32)
        sh = sb.tile([P, T], f32)
        for t in range(T):
            tt = work.tile([P, N], f32)
            nc.scalar.activation(out=tt[:], in_=upoi[:, t, :], func=Act.Relu,
                                 bias=bm[:, t:t + 1], scale=-1.0)
            le = work.tile([P, N], f32)
            nc.vector.tensor_scalar(out=le[:], in0=tt[:],
                                    scalar1=float(MARGIN), scalar2=None,
                                    op0=Alu.is_le)
            sm = work.tile([P, N], f32)
            nc.vector.tensor_tensor(out=sm[:], in0=tt[:], in1=le[:],
                                    op=Alu.mult)
            nc.vector.tensor_reduce(out=pps[:, t:t + 1], in_=sm[:],
                                    axis=Ax.X, op=Alu.max)
            g10 = work.tile([P, N], f32)
            nc.vector.tensor_scalar(out=g10[:], in0=tt[:],
                                    scalar1=float(MARGIN), scalar2=-RHO,
                                    op0=Alu.is_gt, op1=Alu.mult)
            hh = work.tile([P, N], f32)
            nc.vector.tensor_tensor(out=hh[:], in0=tt[:], in1=g10[:],
                                    op=Alu.add)
            nc.vector.tensor_reduce(out=sh[:, t:t + 1], in_=hh[:],
                                    axis=Ax.X, op=Alu.min)

        # ---- combine: ppl = w * (pps + (pps==0)*(sh+RHO)*(sh<0)) -------
        ez = sb.tile([P, T], f32)
        nc.vector.tensor_scalar(out=ez[:], in0=pps[:], scalar1=0.0,
                                scalar2=None, op0=Alu.is_equal)
        sneg = sb.tile([P, T], f32)
        nc.vector.tensor_scalar(out=sneg[:], in0=sh[:], scalar1=0.0,
                                scalar2=None, op0=Alu.is_lt)
        pph = sb.tile([P, T], f32)
        nc.vector.tensor_scalar(out=pph[:], in0=sh[:], scalar1=RHO,
                                scalar2=None, op0=Alu.add)
        nc.vector.tensor_tensor(out=pph[:], in0=pph[:], in1=sneg[:],
                                op=Alu.mult)
        nc.vector.tensor_tensor(out=pph[:], in0=pph[:], in1=ez[:],
                                op=Alu.mult)
        ppl = sb.tile([P, T], f32)
        nc.vector.tensor_tensor(out=ppl[:], in0=pps[:], in1=pph[:],
                                op=Alu.add)
        nc.vector.tensor_tensor(out=ppl[:], in0=ppl[:], in1=wv[:],
                                op=Alu.mult)
        pcol = sb.tile([P, 1], f32)
        nc.vector.tensor_reduce(out=pcol[:], in_=ppl[:], axis=Ax.X,
                                op=Alu.add)
        ps_out = ps.tile([1, 1], f32)
        nc.tensor.matmul(out=ps_out[:], lhsT=pcol[:, 0:1], rhs=ones[:, 0:1],
                         start=True, stop=True)
        osb = sb.tile([1, 1], f32)
        nc.scalar.copy(out=osb[:], in_=ps_out[:])
        nc.sync.dma_start(out=out[:, :], in_=osb[:])

    nc.compile()
    return nc


_last_results = None  # stashed BassKernelResults for profiling in test.py


def kernel(classes: np.ndarray, embeddings: np.ndarray) -> np.ndarray:
    global _last_results
    from concourse import bass_utils

    cls = np.asarray(classes).astype(np.int64)
    emb = np.ascontiguousarray(np.asarray(embeddings), dtype=np.float32)
    assert emb.shape == (N, D)

    same = cls[:, None] == cls[None, :]
    same_nd = same.copy()
    np.fill_diagonal(same_nd, False)
    ii, jj = np.nonzero(same_nd)  # valid (anchor, positive) pairs
    den = len(ii)
    if den == 0:
        return np.asarray(0.0, dtype=np.float32)

    T = max(1, math.ceil(den / (NCORES * P)))
    NI = T * P
    NIC = (NI + 15) // 16
    nslots = NCORES * NI
    ri = np.zeros(nslots, np.int64)
    bo = np.zeros(nslots, np.int32)
    wv = np.zeros(nslots, np.float32)
    ri[:den] = ii
    bo[:den] = (ii * N + jj).astype(np.int32)
    wv[:den] = 1.0

    nc = _build(T)
    in_maps = []
    for c in range(NCORES):
        sl = slice(c * NI, (c + 1) * NI)
        ric, boc, wvc = ri[sl], bo[sl], wv[sl]
        # poison rows, slot n = t*128+p -> poirows[p, t*N + k]
        poir = (BIG * (cls[ric][:, None] == cls[None, :])).astype(np.float32)
        poir = np.ascontiguousarray(
            poir.reshape(T, P, N).transpose(1, 0, 2).reshape(P, T * N))
        in_maps.append({
            "emb": emb,
            "rowidx": np.ascontiguousarray(ric.astype(np.int32).reshape(T, P).T),
            "boff": np.ascontiguousarray(boc.reshape(T, P).T),
            "wvec": np.ascontiguousarray(wvc.reshape(T, P).T),
            "poirows": poir,
        })

    res = bass_utils.run_bass_kernel_spmd(nc, in_maps,
                                          core_ids=list(range(NCORES)))
    _last_results = res
    num = float(sum(r["partial"][0, 0] for r in res.results))
    loss = num / max(den, 1)
    return np.asarray(loss, dtype=np.float32)


# revision 7
# speedup vs baseline: 1.2708x; 1.2708x over previous
"""FacenetLoss Trainium2 kernel.

Strategy (v4)
-------------
N=384, D=128.  The reference builds an [N,N,N] triplet tensor; we never
materialize it.  For anchor i, positive j, negative k:

    tl[i,j,k] = relu((d_ij + MARGIN) - d_ik)

with d the squared-euclidean pairwise distance.  Since
d_ij - d_ik = (sq_j - 2 G_ij) - (sq_k - 2 G_ik)  (sq_i cancels, G = E @ E.T),
we compute on device:

    U[i,k] = sq_k - 2*G[i,k]          (bias source: bias(i,j) = U[i,j] + M)
    V[i,k] = -U[i,k] - poison[i,k]    (row source;  poison = BIG where
                                       classes equal -> invalid negatives k
                                       give tl == 0 exactly)

Only same-class (i,j) pairs ever contribute to the loss (rows with
same_group == 0 reduce to exactly 0 in the reference), so we enumerate the
~N*N/C valid pairs on host (integer bookkeeping only; all math derived from
embeddings stays on device), pack them 128 per partition-tile, gather
V[i,:] / U[i*N+j] per pair via indirect DMA, and per pair-row over k:

    t    = relu(V_row + bias)                                  [ACT]
    pps  = max_k( t * (t <= M) )                               [DVE]
    sh   = min_k( t - RHO*(t > M) )                            [DVE]
    ppl  = w * ( pps + (pps==0) * (sh+RHO) * (sh<0) )          [DVE, small]

Per-core scalar partials are summed on host; loss = num / max(den,1) with den
the host-side valid-pair count.  Poisoned k give t == 0 exactly, which drops
out of pps (max of nonnegatives) and leaves the (sh<0)-gated pph unchanged.
"""

import functools
import math

import numpy as np

N = 384
D = 128
NB = N // 128  # 3 row blocks
P = 128
NCORES = 8
MARGIN = 0.2
RHO = 10.0
BIG = 1.0e5


@functools.lru_cache(maxsize=4)
def _build(T: int):
    """Compile the SPMD kernel for T pair-tiles per core. Returns nc."""
    from contextlib import ExitStack

    import concourse.bacc as bacc
    import concourse.bass as bass
    import concourse.mybir as mybir
    import concourse.tile as tile
    from concourse.masks import make_identity

    f32 = mybir.dt.float32
    i32 = mybir.dt.int32
    Alu = mybir.AluOpType
    Act = mybir.ActivationFunctionType
    Ax = mybir.AxisListType

    nc = bacc.Bacc("TRN2", target_bir_lowering=False, debug=False,
                   num_devices=NCORES)

    emb = nc.dram_tensor("emb", [N, D], f32, kind="ExternalInput").ap()
    poison = nc.dram_tensor("poison", [N, N], f32, kind="ExternalInput").ap()
    rowidx = nc.dram_tensor("rowidx", [P, T], i32, kind="ExternalInput").ap()
    boff = nc.dram_tensor("boff", [P, T], i32, kind="ExternalInput").ap()
    wvec = nc.dram_tensor("wvec", [P, T], f32, kind="ExternalInput").ap()
    out = nc.dram_tensor("partial", [1, 1], f32, kind="ExternalOutput").ap()

    with tile.TileContext(nc) as tc, ExitStack() as ctx:
        sb = ctx.enter_context(tc.tile_pool(name="sb", bufs=1))
        work = ctx.enter_context(tc.tile_pool(name="work", bufs=3))
        ps = ctx.enter_context(tc.tile_pool(name="ps", bufs=2, space="PSUM"))
        dram = ctx.enter_context(tc.tile_pool(name="dram", bufs=1,
                                              space="DRAM"))

        # ---- Phase A: U = sqrow - 2G and V = -U - poison, to DRAM --------
        esb = sb.tile([P, NB * D], f32)
        for b in range(NB):
            nc.sync.dma_start(out=esb[:, b * D:(b + 1) * D],
                              in_=emb[b * P:(b + 1) * P, :])

        idt = sb.tile([P, P], f32)
        make_identity(nc, idt[:])

        # poison blocks + host tables (emitted after identity: gpsimd order)
        poi = sb.tile([P, NB, N], f32)
        for b in range(NB):
            nc.sync.dma_start(out=poi[:, b, :],
                              in_=poison[b * P:(b + 1) * P, :])
        ridx = sb.tile([P, T], i32)
        nc.gpsimd.dma_start(out=ridx[:], in_=rowidx[:, :])
        bofs = sb.tile([P, T], i32)
        nc.gpsimd.dma_start(out=bofs[:], in_=boff[:, :])
        wv = sb.tile([P, T], f32)
        nc.gpsimd.dma_start(out=wv[:], in_=wvec[:, :])

        # ET[d, n] = emb[n, d]  via PE transpose per block
        et = sb.tile([P, N], f32)
        for b in range(NB):
            pst = ps.tile([P, P], f32)
            nc.tensor.transpose(out=pst[:], in_=esb[:, b * D:(b + 1) * D],
                                identity=idt[:])
            nc.scalar.copy(out=et[:, b * P:(b + 1) * P], in_=pst[:])

        # sqrow[0, n] = sum_d emb[n, d]^2 ; broadcast to all partitions
        etsq = sb.tile([P, N], f32)
        nc.vector.tensor_tensor(out=etsq[:], in0=et[:], in1=et[:],
                                op=Alu.mult)
        ones = sb.tile([P, 1], f32)
        nc.vector.memset(ones[:], 1.0)
        ps_sq = ps.tile([1, N], f32)
        nc.tensor.matmul(out=ps_sq[:], lhsT=ones[:, 0:1], rhs=etsq[:],
                         start=True, stop=True)
        sqrow = sb.tile([1, N], f32)
        nc.scalar.copy(out=sqrow[:], in_=ps_sq[:])
        sqb = sb.tile([P, N], f32)
        nc.gpsimd.partition_broadcast(sqb[:], sqrow[:])
        etm2 = sb.tile([P, N], f32)
        nc.vector.tensor_scalar(out=etm2[:], in0=et[:], scalar1=-2.0,
                                scalar2=None, op0=Alu.mult)

        udram = dram.tile([N * N, 1], f32)
        ud2 = udram[:, :].rearrange("(a b) o -> a (b o)", b=N)
        vdram = dram.tile([N, N], f32)
        for b in range(NB):
            ps_g = ps.tile([P, N], f32)
            nc.tensor.matmul(out=ps_g[:], lhsT=etm2[:, b * P:(b + 1) * P],
                             rhs=et[:], start=True, stop=True)  # -2G
            ub = work.tile([P, N], f32)
            nc.vector.tensor_tensor(out=ub[:], in0=ps_g[:], in1=sqb[:],
                                    op=Alu.add)
            nc.sync.dma_start(out=ud2[b * P:(b + 1) * P, :], in_=ub[:])
            vb = work.tile([P, N], f32)
            nc.vector.tensor_scalar(out=vb[:], in0=ub[:], scalar1=-1.0,
                                    scalar2=None, op0=Alu.mult)
            nc.vector.tensor_tensor(out=vb[:], in0=vb[:], in1=poi[:, b, :],
                                    op=Alu.subtract)
            nc.sync.dma_start(out=vdram[b * P:(b + 1) * P, :], in_=vb[:])

        # ---- Phase B: per-tile gathers + triplet reductions -------------
        bval = work.tile([P, T], f32)
        vrows = work.tile([P, T, N], f32)
        for t in range(T):
            nc.gpsimd.indirect_dma_start(
                out=vrows[:, t, :], out_offset=None, in_=vdram[:, :],
                in_offset=bass.IndirectOffsetOnAxis(ap=ridx[:, t:t + 1],
                                                    axis=0))
            nc.gpsimd.indirect_dma_start(
                out=bval[:, t:t + 1], out_offset=None, in_=udram[:, :],
                in_offset=bass.IndirectOffsetOnAxis(ap=bofs[:, t:t + 1],
                                                    axis=0))

        pps = sb.tile([P, T], f32)
        sh = sb.tile([P, T], f32)
        for t in range(T):
            bm = work.tile([P, 1], f32)
            nc.vector.tensor_scalar(out=bm[:], in0=bval[:, t:t + 1],
                                    scalar1=float(MARGIN), scalar2=None,
                                    op0=Alu.add)
            tt = work.tile([P, N], f32)
            nc.scalar.activation(out=tt[:], in_=vrows[:, t, :],
                                 func=Act.Relu, bias=bm[:, 0:1], scale=1.0)
            le = work.tile([P, N], f32)
            nc.vector.tensor_scalar(out=le[:], in0=tt[:],
                                    scalar1=float(MARGIN), scalar2=None,
                                    op0=Alu.is_le)
            sm = work.tile([P, N], f32)
            nc.vector.tensor_tensor(out=sm[:], in0=tt[:], in1=le[:],
                                    op=Alu.mult)
            nc.vector.tensor_reduce(out=pps[:, t:t + 1], in_=sm[:],
                                    axis=Ax.X, op=Alu.max)
            g10 = work.tile([P, N], f32)
            nc.vector.tensor_scalar(out=g10[:], in0=tt[:],
                                    scalar1=float(MARGIN), scalar2=-RHO,
                                    op0=Alu.is_gt, op1=Alu.mult)
            hh = work.tile([P, N], f32)
            nc.vector.tensor_tensor(out=hh[:], in0=tt[:], in1=g10[:],
                                    op=Alu.add)
            nc.vector.tensor_reduce(out=sh[:, t:t + 1], in_=hh[:],
                                    axis=Ax.X, op=Alu.min)

        # ---- combine: ppl = w * (pps + (pps==0)*(sh+RHO)*(sh<0)) -------
        ez = sb.tile([P, T], f32)
        nc.vector.tensor_scalar(out=ez[:], in0=pps[:], scalar1=0.0,
                                scalar2=None, op0=Alu.is_equal)
        sneg = sb.tile([P, T], f32)
        nc.vector.tensor_scalar(out=sneg[:], in0=sh[:], scalar1=0.0,
                                scalar2=None, op0=Alu.is_lt)
        pph = sb.tile([P, T], f32)
        nc.vector.tensor_scalar(out=pph[:], in0=sh[:], scalar1=RHO,
                                scalar2=None, op0=Alu.add)
        nc.vector.tensor_tensor(out=pph[:], in0=pph[:], in1=sneg[:],
                                op=Alu.mult)
        nc.vector.tensor_tensor(out=pph[:], in0=pph[:], in1=ez[:],
                                op=Alu.mult)
        ppl = sb.tile([P, T], f32)
        nc.vector.tensor_tensor(out=ppl[:], in0=pps[:], in1=pph[:],
                                op=Alu.add)
        nc.vector.tensor_tensor(out=ppl[:], in0=ppl[:], in1=wv[:],
                                op=Alu.mult)
        pcol = sb.tile([P, 1], f32)
        nc.vector.tensor_reduce(out=pcol[:], in_=ppl[:], axis=Ax.X,
                                op=Alu.add)
        ps_out = ps.tile([1, 1], f32)
        nc.tensor.matmul(out=ps_out[:], lhsT=pcol[:, 0:1], rhs=ones[:, 0:1],
                         start=True, stop=True)
        osb = sb.tile([1, 1], f32)
        nc.scalar.copy(out=osb[:], in_=ps_out[:])
        nc.sync.dma_start(out=out[:, :], in_=osb[:])

    nc.compile()
    return nc


_last_results = None  # stashed BassKernelResults for profiling in test.py


def kernel(classes: np.ndarray, embeddings: np.ndarray) -> np.ndarray:
    global _last_results
    from concourse import bass_utils

    cls = np.asarray(classes).astype(np.int64)
    emb = np.ascontiguousarray(np.asarray(embeddings), dtype=np.float32)
    assert emb.shape == (N, D)

    same = cls[:, None] == cls[None, :]
    poison = (BIG * same).astype(np.float32)
    same_nd = same.copy()
    np.fill_diagonal(same_nd, False)
    ii, jj = np.nonzero(same_nd)  # valid (anchor, positive) pairs
    den = len(ii)
    if den == 0:
        return np.asarray(0.0, dtype=np.float32)

    T = max(1, math.ceil(den / (NCORES * P)))
    NI = T * P
    nslots = NCORES * NI
    ri = np.zeros(nslots, np.int64)
    bo = np.zeros(nslots, np.int32)
    wv = np.zeros(nslots, np.float32)
    ri[:den] = ii
    bo[:den] = (ii * N + jj).astype(np.int32)
    wv[:den] = 1.0

    nc = _build(T)
    in_maps = []
    for c in range(NCORES):
        sl = slice(c * NI, (c + 1) * NI)
        ric, boc, wvc = ri[sl], bo[sl], wv[sl]
        in_maps.append({
            "emb": emb,
            "poison": poison,
            "rowidx": np.ascontiguousarray(ric.astype(np.int32).reshape(T, P).T),
            "boff": np.ascontiguousarray(boc.reshape(T, P).T),
            "wvec": np.ascontiguousarray(wvc.reshape(T, P).T),
        })

    res = bass_utils.run_bass_kernel_spmd(nc, in_maps,
                                          core_ids=list(range(NCORES)))
    _last_results = res
    num = float(sum(r["partial"][0, 0] for r in res.results))
    loss = num / max(den, 1)
    return np.asarray(loss, dtype=np.float32)
